# revision 1
# baseline (speedup 1.0000x reference)
"""Trainium2 Bass kernel for additive-attention pooling.

Reference math (per sample b):
    score  = tanh(x @ W_w + W_b)          # [T, U]
    logits = score @ V_w + V_b            # [T, 1]
    attn   = softmax(logits, axis=T)
    out    = sum_t attn[t] * x[t, :]      # [D]

V_b shifts every logit of a sample equally, so it cancels in the softmax and
is dropped. Softmax max-subtraction is skipped: |logit| <= sum|V| < 20,
safely inside fp32/bf16 exp range.

Sharding: data-parallel over batch, 8 samples per core on 8 NeuronCores,
gathered on the host. All heavy compute in bf16 (rel err ~2e-3 vs fp32).

The host ships x twice (both bf16 -- same total bytes as one fp32 copy):
pre-transposed [D, T] for the score GEMM (TensorE contracts over the
partition axis, so the GEMM needs d-on-partitions) and natural [T, D+1]
with a ones column for the softmax-weighted sum (t-on-partitions; the ones
column makes the softmax denominator fall out of the same matmul). This
avoids all on-chip transposes.

Per 512-row tile, software-pipelined with Tile-framework semaphores:
  1. GEMM (TensorE): score^T [u, t] = W-block^T @ xT, N=512 matmuls,
     W blocks kept stationary across a pair of tiles. Both input streams
     ride one DMA queue with the GEMM-critical xT transfers issued first.
  2. tanh (ScalarE): PSUM -> SBUF bf16, per-partition bias = W_b chunk.
  3. V-fold (VectorE): z = V0*tanh_u0 + V1*tanh_u1 with per-partition
     scalars, halving the TensorE V-dot work (lag 2 tiles).
  4. V-dot (TensorE): single-matmul reduce per 128-row chunk
     (lhsT = z chunk, rhs = ones) -> logits land [t, 1]-shaped, exactly
     the stationary layout the weighted sum needs. These N=1 matmuls are
     interleaved between long matmuls so their LDWEIGHTS stay hidden.
  5. exp (ScalarE): one [128, 32] activation per sample.
  6. weighted sum (TensorE): lhsT = exp-weight column, rhs = xn chunk
     (N=257), accumulated in PSUM over the sample; lags one sample behind
     the GEMM pipeline so it never waits on exp.
  7. finalize (VectorE): context = c[0:256] * (1/c[256]), DMA out.
"""

import numpy as np
import ml_dtypes

# ---- problem constants (hardcoded; kernel.py must be self-contained) ----
B, T, D, U = 64, 4096, 256, 256
N_CORES = 8
S = B // N_CORES          # samples per core
TT = 512                  # t-tile (rows per pipeline step)
N_TILES = T // TT         # tiles per sample (8)
CH = TT // 128            # 128-row chunks per tile (4)
LAG_L2 = 2                # V-dot lag in tiles (z computed on VectorE)
LAG_W = N_TILES + 2       # weighted-sum lag in tiles

BF16 = ml_dtypes.bfloat16

_CACHE = {}


def _build():
    import concourse.bass as bass
    import concourse.tile as tile
    from concourse import bacc, mybir
    from concourse.bass import ds, ts

    f32 = mybir.dt.float32
    bf16 = mybir.dt.bfloat16
    Tanh = mybir.ActivationFunctionType.Tanh
    Exp = mybir.ActivationFunctionType.Exp

    nc = bacc.Bacc("TRN2", target_bir_lowering=False, debug=False)

    xT_d = nc.dram_tensor("xT", [S, D, T], bf16, kind="ExternalInput").ap()
    xn_d = nc.dram_tensor("xn", [S, T, D + 1], bf16, kind="ExternalInput").ap()
    w_d = nc.dram_tensor("w", [D, U], bf16, kind="ExternalInput").ap()
    wb_d = nc.dram_tensor("wb", [128, U // 128], f32, kind="ExternalInput").ap()
    v_d = nc.dram_tensor("v", [128, U // 128], f32, kind="ExternalInput").ap()
    out_d = nc.dram_tensor("out", [S, D], f32, kind="ExternalOutput").ap()

    NG = S * N_TILES  # total pipeline steps (64)

    with tile.TileContext(nc) as tc:
        with (
            tc.tile_pool(name="const", bufs=1) as const_pool,
            tc.tile_pool(name="xT", bufs=10) as xT_pool,
            tc.tile_pool(name="xn", bufs=LAG_W + 3) as xn_pool,
            tc.tile_pool(name="tanh", bufs=3) as tanh_pool,
            tc.tile_pool(name="z", bufs=6) as z_pool,
            tc.tile_pool(name="wexp", bufs=2) as wexp_pool,
            tc.tile_pool(name="fin", bufs=2) as fin_pool,
            tc.tile_pool(name="score_ps", bufs=6, space="PSUM") as score_pool,
            tc.tile_pool(name="logit_ps", bufs=1, space="PSUM") as logit_pool,
            tc.tile_pool(name="c_ps", bufs=1, space="PSUM") as c_pool,
        ):
            # constants
            w_sb = const_pool.tile([128, 2, U], bf16)     # [d_in_chunk, d_chunk, u]
            nc.gpsimd.dma_start(w_sb[:], w_d.rearrange("(k p) u -> p k u", p=128))
            v_sb = const_pool.tile([128, 2], f32)         # [u_in_chunk, u_chunk]
            nc.gpsimd.dma_start(v_sb[:], v_d)
            wb_sb = const_pool.tile([128, 2], f32)
            nc.gpsimd.dma_start(wb_sb[:], wb_d)
            ones_sb = const_pool.tile([128, 1], bf16)
            nc.vector.memset(ones_sb[:], 1.0)

            z_tiles = {}        # g -> V-reduced tanh tile [128, TT]
            score_tiles = {}    # (g, uc) -> psum score tile
            xn_tiles = {}       # g -> xn tile
            logit_tiles = {}    # sample -> [128, N_TILES*CH] psum tile
            wexp_tiles = {}     # sample -> [128, N_TILES*CH] bf16 weights
            c_tiles = {}        # sample -> [1, D+1] psum accumulator

            def emit_l2(j, c):
                """Partition-reduce of z chunk c for tile j -> logit column."""
                sj, ttj = divmod(j, N_TILES)
                nc.tensor.matmul(
                    logit_tiles[sj][:, ds(ttj * CH + c, 1)],
                    z_tiles[j][:, ts(c, 128)],
                    ones_sb[:],
                    start=True,
                    stop=True,
                )
                if c == CH - 1:
                    del z_tiles[j]

            def emit_wsum_chunk(j, c):
                """One 128-row chunk of the weighted sum for tile j."""
                sj, ttj = divmod(j, N_TILES)
                nc.tensor.matmul(
                    c_tiles[sj][:],
                    wexp_tiles[sj][:, ds(ttj * CH + c, 1)],
                    xn_tiles[j][:, c, :],
                    start=(ttj == 0 and c == 0),
                    stop=(ttj == N_TILES - 1 and c == CH - 1),
                )
                if c == CH - 1:
                    del xn_tiles[j]

            for g in range(NG + LAG_W + 1):
                s, tt = divmod(g, N_TILES) if g < NG else (None, None)
                jw = g - LAG_W  # tile index for weighted sum this iteration
                jl = g - LAG_L2  # tile index for V-dot this iteration

                # ---- DMA + paired GEMM (W block stationary reused) ----
                if g < NG and g % 2 == 0:
                    pair = [g, g + 1]
                    xt_pair = []
                    for gg in pair:
                        ss, tts = divmod(gg, N_TILES)
                        xT_t = xT_pool.tile([128, 2, TT], bf16, tag="xT", name=f"xT{gg}")
                        nc.sync.dma_start(
                            xT_t[:],
                            xT_d[ss, :, ts(tts, TT)].rearrange(
                                "(k p) t -> p k t", p=128
                            ),
                        )
                        xt_pair.append(xT_t)
                    for gg in pair:
                        ss, tts = divmod(gg, N_TILES)
                        xn_t = xn_pool.tile([128, CH, D + 1], bf16, tag="xn", name=f"xn{gg}")
                        nc.sync.dma_start(
                            xn_t[:],
                            xn_d[ss, ts(tts, TT), :].rearrange(
                                "(c p) f -> p c f", p=128
                            ),
                        )
                        xn_tiles[gg] = xn_t
                        if tts == 0:
                            logit_tiles[ss] = logit_pool.tile(
                                [128, N_TILES * CH], f32, tag="logit",
                                name=f"logit{ss}",
                            )
                            c_tiles[ss] = c_pool.tile(
                                [1, D + 1], f32, tag="acc", name=f"acc{ss}"
                            )
                    for uc in range(2):
                        scs = [
                            score_pool.tile(
                                [128, TT], f32, tag="score", name=f"sc{gg}_{uc}"
                            )
                            for gg in pair
                        ]
                        for dc in range(2):
                            for pi in range(2):
                                nc.tensor.matmul(
                                    scs[pi][:],
                                    w_sb[:, dc, ts(uc, 128)],
                                    xt_pair[pi][:, dc, :],
                                    start=(dc == 0),
                                    stop=(dc == 1),
                                )
                        score_tiles[(pair[0], uc)] = scs[0]
                        score_tiles[(pair[1], uc)] = scs[1]

                # ---- tail matmuls: L2 + wsum ----
                li = 0
                n_l2 = CH if 0 <= jl < NG else 0
                if 0 <= jw < NG:
                    for c in range(CH):
                        emit_wsum_chunk(jw, c)
                        if li < n_l2:
                            emit_l2(jl, li)
                            li += 1
                while li < n_l2:
                    emit_l2(jl, li)
                    li += 1

                # ---- ACT: exp once per sample (after last tile's V-dot) ----
                if 0 <= jl < NG and jl % N_TILES == N_TILES - 1:
                    sj = jl // N_TILES
                    lg = logit_tiles.pop(sj)
                    wx = wexp_pool.tile([128, N_TILES * CH], bf16, tag="wexp")
                    nc.scalar.activation(wx[:], lg[:], Exp)
                    wexp_tiles[sj] = wx

                # ---- ACT: tanh; DVE: fold V into the two u-halves ----
                if g < NG:
                    tanh_t = tanh_pool.tile([128, 2, TT], bf16)
                    for uc in range(2):
                        nc.scalar.activation(
                            tanh_t[:, uc, :],
                            score_tiles.pop((g, uc))[:],
                            Tanh,
                            bias=wb_sb[:, ds(uc, 1)],
                        )
                    q = z_pool.tile([128, TT], bf16, tag="q")
                    nc.vector.tensor_scalar_mul(q[:], tanh_t[:, 0, :], v_sb[:, ds(0, 1)])
                    zt = z_pool.tile([128, TT], bf16, tag="z")
                    nc.vector.tensor_scalar_mul(zt[:], tanh_t[:, 1, :], v_sb[:, ds(1, 1)])
                    nc.vector.tensor_add(zt[:], zt[:], q[:])
                    z_tiles[g] = zt

                # ---- finalize sample after its last wsum chunk ----
                if 0 <= jw < NG and jw % N_TILES == N_TILES - 1:
                    sj = jw // N_TILES
                    del wexp_tiles[sj]
                    c_ps = c_tiles.pop(sj)
                    recip = fin_pool.tile([1, 1], f32, tag="recip")
                    nc.vector.reciprocal(recip[:], c_ps[0:1, D : D + 1])
                    row = fin_pool.tile([1, D], f32, tag="row")
                    nc.vector.tensor_scalar_mul(row[:], c_ps[0:1, 0:D], recip[:])
                    nc.scalar.dma_start(out_d[ds(sj, 1), :], row[:])

    nc.compile()
    return nc


def _prep_inputs(inputs, W_w, W_b, V_w, V_b):
    x = np.asarray(inputs, dtype=np.float32)
    ones = np.ones((B, T, 1), dtype=np.float32)
    xn_full = np.concatenate([x, ones], axis=2).astype(BF16)      # [B, T, D+1]
    xT_full = np.ascontiguousarray(x.transpose(0, 2, 1)).astype(BF16)  # [B, D, T]

    w = np.asarray(W_w, dtype=np.float32).astype(BF16)            # [D, U]
    wb = np.asarray(W_b, dtype=np.float32).reshape(U // 128, 128).T.copy()  # [128, 2]
    v = np.asarray(V_w, dtype=np.float32).reshape(U // 128, 128).T.copy()  # [128, 2]

    in_maps = []
    for c in range(N_CORES):
        sl = slice(c * S, (c + 1) * S)
        in_maps.append(
            {
                "xT": np.ascontiguousarray(xT_full[sl]),
                "xn": np.ascontiguousarray(xn_full[sl]),
                "w": w,
                "wb": wb,
                "v": v,
            }
        )
    return in_maps


def kernel(inputs, W_w, W_b, V_w, V_b):
    from concourse.bass_utils import run_bass_kernel_spmd

    if "nc" not in _CACHE:
        _CACHE["nc"] = _build()
    nc = _CACHE["nc"]

    in_maps = _prep_inputs(inputs, W_w, W_b, V_w, V_b)
    res = run_bass_kernel_spmd(nc, in_maps, core_ids=list(range(N_CORES)))
    out = np.concatenate([r["out"] for r in res.results], axis=0)
    return np.asarray(out, dtype=np.float32)

